# revision 14
# baseline (speedup 1.0000x reference)
"""Trainium2 Bass kernel for nn_CrossAttention2D.

Reference computation (per batch b, row h):
    Q = w1 @ Xw + b1          (Xw = waveform[b,:,h,:]  [C=128, W=512])
    K = w2 @ Xs + b2          (Xs = spectrogram[b,:,h,:])
    S = Q^T K * 1/sqrt(F)     [512, 512]
    P = softmax(S, axis=-1)
    out[b,:,h,:] = Xs @ P^T   [C, W]

Device algorithm: compute S TRANSPOSED (k on partitions) via the
associativity decomposition
    S^T = (M Xs)^T Xw + v 1^T      (M = w1^T w2, v = Xs^T (w2^T b1))
Terms of S constant along k (the softmax axis) — u[q] = Xw^T w1^T b2 and
gamma = b1.b2 — cancel in the softmax and are dropped. With k on
partitions, v[k] is a per-partition ACT bias (free), softmax
normalization becomes
    out = (Xs E^T) * (1 ⊗ 1/r),  E = exp(S^T * s), r = 1^T E
so only Xs needs a PE transpose (4 blocks/row instead of the 20 the
S-orientation requires). 1/r is broadcast with a gpsimd
partition_broadcast and a fast-approx DVE reciprocal.

All matmul operands are bf16 (converted host-side, so no on-device
convert passes and half the input DMA traffic); PSUM accumulation stays
fp32. Measured end-to-end relative error ~8e-3 vs the 2e-2 gate.

Sharding: data-parallel over batch B=8 across 8 NeuronCores (one batch
image per core, small weights replicated). No collectives.
"""

import contextlib

import numpy as np
import ml_dtypes

import concourse.bacc as bacc
import concourse.tile as tile
from concourse import mybir
from concourse.bass_utils import run_bass_kernel_spmd

B = 8
C = 128  # channel dim (TIME_DIM == SPEC_DIM == 128)
H = 64
W = 512
N_CORES = 8
SCALE = 1.0 / 16.0  # 1/sqrt(FEATURE_DIM=256)

FP32 = mybir.dt.float32
BF16 = mybir.dt.bfloat16
EXP = mybir.ActivationFunctionType.Exp
IDENT = mybir.ActivationFunctionType.Identity


def build_module(n_h=H, rep=1):
    """Build the per-core Bass module processing [C, n_h, W] inputs.

    rep > 1 repeats the computation on the same data (timing runs only).
    """
    nc = bacc.Bacc("TRN2", target_bir_lowering=False, debug=False)

    wave = nc.dram_tensor("wave", [C, n_h, W], BF16, kind="ExternalInput").ap()
    spec = nc.dram_tensor("spec", [C, n_h, W], BF16, kind="ExternalInput").ap()
    # mt = (w2^T w1) so that matmul's lhsT.T = w1^T w2 = M
    mt = nc.dram_tensor("mt", [C, C], BF16, kind="ExternalInput").ap()
    # beta2 = w2^T b1  (the K-side bias that survives the softmax)
    beta2 = nc.dram_tensor("beta2", [C, 1], BF16, kind="ExternalInput").ap()
    ident = nc.dram_tensor("ident", [C, C], BF16, kind="ExternalInput").ap()
    out = nc.dram_tensor("out", [C, n_h, W], FP32, kind="ExternalOutput").ap()

    with tile.TileContext(nc) as tc:
        with (
            tc.tile_pool(name="consts", bufs=1) as consts,
            tc.tile_pool(name="io", bufs=6) as io,
            tc.tile_pool(name="work", bufs=4) as work,
            tc.tile_pool(name="small", bufs=6) as small,
            tc.tile_pool(name="ps", bufs=2, space="PSUM") as ps,
            tc.tile_pool(name="pm", bufs=1, space="PSUM") as pm,
            tc.tile_pool(name="po", bufs=2, space="PSUM") as po,
            tc.tile_pool(name="pss", bufs=1, space="PSUM") as pss,
        ):
            mt_sb = consts.tile([C, C], BF16, tag="mt")
            nc.sync.dma_start(mt_sb, mt)
            b2_sb = consts.tile([C, 1], BF16, tag="b2")
            nc.sync.dma_start(b2_sb, beta2)
            id_sb = consts.tile([C, C], BF16, tag="id")
            nc.sync.dma_start(id_sb, ident)
            onec = consts.tile([C, 1], BF16, tag="onec")
            nc.vector.memset(onec, 1.0)

            rep_ctx = tc.For_i(0, rep, 1) if rep > 1 else contextlib.nullcontext()
            with rep_ctx:
                for h in range(n_h):
                    xw = io.tile([C, W], BF16, tag="xw")
                    nc.sync.dma_start(xw, wave[:, h, :])
                    xs = io.tile([C, W], BF16, tag="xs")
                    nc.sync.dma_start(xs, spec[:, h, :])

                    # T = M @ Xs  [c, k]
                    t_ps = pm.tile([C, W], FP32, tag="pm512")
                    nc.tensor.matmul(t_ps, mt_sb, xs, start=True, stop=True)
                    t_sb = work.tile([C, W], BF16, tag="t")
                    nc.vector.tensor_copy(t_sb, t_ps)

                    # v[k] = Xs^T beta2, per 128-k chunk
                    v_ps = pss.tile([C, 4], FP32, tag="vps")
                    for kc in range(4):
                        nc.tensor.matmul(
                            v_ps[:, kc : kc + 1],
                            xs[:, kc * 128 : (kc + 1) * 128],
                            b2_sb,
                            start=True,
                            stop=True,
                        )
                    v_sb = small.tile([C, 4], FP32, tag="v")
                    nc.vector.tensor_scalar_mul(v_sb, v_ps, SCALE)

                    # Xs^T blocks (needed as lhsT of the output matmul)
                    xst_ps = pm.tile([C, W], BF16, tag="xstp")
                    for kc in range(4):
                        nc.tensor.transpose(
                            xst_ps[:, kc * 128 : (kc + 1) * 128],
                            xs[:, kc * 128 : (kc + 1) * 128],
                            id_sb,
                        )
                    xst_sb = work.tile([C, W], BF16, tag="xst")
                    nc.vector.tensor_copy(xst_sb, xst_ps)

                    # S^T chunks [128k, 512q] + exp with per-partition bias
                    e_sb = work.tile([C, 4, W], BF16, tag="e")
                    for kc in range(4):
                        s_ps = ps.tile([C, W], FP32, tag="ps512")
                        nc.tensor.matmul(
                            s_ps,
                            t_sb[:, kc * 128 : (kc + 1) * 128],
                            xw,
                            start=True,
                            stop=True,
                        )
                        nc.scalar.activation(
                            e_sb[:, kc, :],
                            s_ps,
                            EXP,
                            bias=v_sb[:, kc : kc + 1],
                            scale=SCALE,
                        )

                    # r[q] = sum_k E^T[k, q]  (accumulate 4 ones-matmuls)
                    r_ps = pss.tile([1, W], FP32, tag="rps")
                    for kc in range(4):
                        nc.tensor.matmul(
                            r_ps,
                            onec,
                            e_sb[:, kc, :],
                            start=(kc == 0),
                            stop=(kc == 3),
                        )

                    # out_unnorm = Xs @ E  (accumulate over k chunks)
                    o_ps = po.tile([C, W], FP32, tag="ops")
                    for kc in range(4):
                        nc.tensor.matmul(
                            o_ps,
                            xst_sb[:, kc * 128 : (kc + 1) * 128],
                            e_sb[:, kc, :],
                            start=(kc == 0),
                            stop=(kc == 3),
                        )

                    # 1/r broadcast: r row -> SBUF (ACT/DVE alternating),
                    # gpsimd broadcast to 128 partitions, fast-approx
                    # reciprocal, then normalize.
                    r_row = small.tile([1, W], FP32, tag="rrow")
                    if h % 2 == 0:
                        nc.scalar.copy(r_row, r_ps)
                    else:
                        nc.vector.tensor_copy(r_row, r_ps)
                    rbc = work.tile([C, W], FP32, tag="rbc")
                    nc.gpsimd.partition_broadcast(rbc, r_row)
                    rinv = work.tile([C, W], FP32, tag="rinv")
                    nc.vector.reciprocal_approx_fast(rinv, rbc)

                    o_sb = io.tile([C, W], FP32, tag="o")
                    nc.vector.tensor_mul(o_sb, o_ps, rinv)
                    nc.sync.dma_start(out[:, h, :], o_sb)

    nc.compile()
    return nc


def host_prep(w1, b1, w2, b2):
    """Precompute the small host-side tensors (float64 for accuracy)."""
    w1d = np.asarray(w1, np.float64)
    w2d = np.asarray(w2, np.float64)
    b1d = np.asarray(b1, np.float64)
    mt = np.ascontiguousarray((w2d.T @ w1d).astype(ml_dtypes.bfloat16))
    beta2 = np.ascontiguousarray((w2d.T @ b1d)[:, None].astype(ml_dtypes.bfloat16))
    ident = np.eye(C, dtype=ml_dtypes.bfloat16)
    return mt, beta2, ident


_NC_CACHE = {}


def _get_nc(n_h=H, rep=1):
    key = (n_h, rep)
    if key not in _NC_CACHE:
        _NC_CACHE[key] = build_module(n_h, rep)
    return _NC_CACHE[key]


def run_device(waveform, spectrogram, w1, b1, w2, b2, n_h=H, rep=1, **run_kwargs):
    """Shard over batch, run on 8 cores, gather. Returns (output, results)."""
    waveform = np.asarray(waveform, np.float32).astype(ml_dtypes.bfloat16)
    spectrogram = np.asarray(spectrogram, np.float32).astype(ml_dtypes.bfloat16)
    mt, beta2, ident = host_prep(w1, b1, w2, b2)

    in_maps = [
        {
            "wave": np.ascontiguousarray(waveform[b, :, :n_h, :]),
            "spec": np.ascontiguousarray(spectrogram[b, :, :n_h, :]),
            "mt": mt,
            "beta2": beta2,
            "ident": ident,
        }
        for b in range(B)
    ]
    nc = _get_nc(n_h, rep)
    res = run_bass_kernel_spmd(nc, in_maps, core_ids=list(range(N_CORES)), **run_kwargs)
    output = np.stack([res.results[b]["out"] for b in range(B)], axis=0)
    return output, res


def kernel(waveform, spectrogram, w1, b1, w2, b2):
    output, _ = run_device(waveform, spectrogram, w1, b1, w2, b2)
    return output.astype(np.float32)
